# revision 10
# baseline (speedup 1.0000x reference)
"""Multi-head attention (B=4, N=2048, D=768, H=12) on 8 TRN2 NeuronCores.

Sharding: batch x head-group. Core c handles batch c//2, heads
[(c%2)*6, (c%2)*6+6). Each core computes qkv projection for its 6 heads
(column-sliced w_qkv), attention, and a partial output projection
(row-sliced w_proj). Host sums the two partial projections per batch and
adds the bias.

Per-core device dataflow (all fp32):
  A: x [NT,768] -> PE-transpose -> xT [128,6,NT] (dim-major)
  B: qkv^T = w_qkv_c^T-slices @ xT -> qkT [128,6,NT] (q:0-2,k:3-5), vT [128,3,NT]
  C: per head h (paired 2 heads per 128-partition tile, base=(h%2)*64):
       v_nat via PE-transpose -> vn [128,KT,128] (v cols at base-half, ones else)
       per q-chunk: S^T tiles = kT_h^T-slices @ qT_h chunk -> psum
                    exp(S*scale) -> es sbuf; O'^T = vn^T @ es (accum 16)
                    po rows base-half = O', other half = row-sums
                    normalize: copy 1 sums row -> sbuf, DMA-broadcast, recip, mul
  D: y = oT^T-slices @ w_proj_c -> psum -> sbuf -> DRAM y [NT,768]
"""

import numpy as np
from contextlib import ExitStack

D = 768
F = 1152          # 3 * 6 heads * 64 per core
HL = 6            # local heads per core
HD = 64
KO = D // 128     # 6 contraction slices for qkv
FT = F // 128     # 9 feature tiles
SCALE = HD ** -0.5
N_CORES = 8
B_FULL, N_FULL = 4, 2048

f32 = None  # set lazily (mybir import is heavy; keep module import cheap)


def build_program(NT=N_FULL, n_cores=N_CORES, repeat=1):
    import concourse.bacc as bacc
    import concourse.tile as tile
    import concourse.mybir as mybir
    from concourse.masks import make_identity

    f32 = mybir.dt.float32
    EXP = mybir.ActivationFunctionType.Exp

    KT = NT // 128            # token tiles
    QC = min(512, NT)         # q-chunk width
    NQC = NT // QC
    SW = min(1024, 2 * QC)    # S^T psum tile width (exp batch)
    KPG = SW // QC            # k-tiles per S psum group
    NSW = KT // KPG           # S psum groups per q-chunk

    nc = bacc.Bacc("TRN2", target_bir_lowering=False, debug=False,
                   enable_asserts=False, num_devices=n_cores)
    x_d = nc.dram_tensor("x", [NT, D], f32, kind="ExternalInput").ap()
    wq_d = nc.dram_tensor("w_qkv", [D, F], f32, kind="ExternalInput").ap()
    wp_d = nc.dram_tensor("w_proj", [HL * HD, D], f32, kind="ExternalInput").ap()
    y_d = nc.dram_tensor("y", [NT, D], f32, kind="ExternalOutput").ap()

    with tile.TileContext(nc) as tc, ExitStack() as ctx:
        constp = ctx.enter_context(tc.tile_pool(name="const", bufs=1))
        ident = constp.tile([128, 128], f32)
        make_identity(nc, ident)
        ones64 = constp.tile([128, 64], f32)
        nc.vector.memset(ones64[:], 1.0)

        actp = ctx.enter_context(tc.tile_pool(name="acts", bufs=1))
        qkT = actp.tile([128, 2 * (HL // 2), NT], f32)   # q tiles 0-2, k tiles 3-5
        vT = actp.tile([128, HL // 2, NT], f32)
        oT = actp.tile([128, HL // 2, NT], f32)

        if repeat > 1:
            rep_cm = tc.For_i(0, repeat, 1)
            rep_cm.__enter__()

        # ---- Phase A: load + transpose x; Phase B: qkv^T ----
        with tc.tile_pool(name="xa", bufs=1) as xap, \
             tc.tile_pool(name="xload", bufs=3) as xlp, \
             tc.tile_pool(name="wq", bufs=1) as wqp, \
             tc.tile_pool(name="ptr", bufs=2, space="PSUM") as ptr, \
             tc.tile_pool(name="pqkv", bufs=4, space="PSUM") as pqkv:
            xT = xap.tile([128, KO, NT], f32)
            wq_sb = wqp.tile([128, KO, F], f32)
            nc.sync.dma_start(
                wq_sb[:], wq_d.rearrange("(ko ki) f -> ki ko f", ki=128))
            for tt in range(KT):
                xt = xlp.tile([128, D], f32, tag="xload")
                nc.sync.dma_start(xt[:], x_d[tt * 128:(tt + 1) * 128, :])
                for ks in range(KO):
                    ps = ptr.tile([128, 128], f32, tag="ptr")
                    nc.tensor.transpose(
                        ps[:], xt[:, ks * 128:(ks + 1) * 128], ident[:])
                    nc.vector.tensor_copy(
                        xT[:, ks, tt * 128:(tt + 1) * 128], ps[:])
            for ft in range(FT):
                for qc in range(NQC):
                    ps = pqkv.tile([128, QC], f32, tag="pqkv")
                    for ks in range(KO):
                        nc.tensor.matmul(
                            ps[:],
                            wq_sb[:, ks, ft * 128:(ft + 1) * 128],
                            xT[:, ks, qc * QC:(qc + 1) * QC],
                            start=(ks == 0), stop=(ks == KO - 1))
                    if ft < 6:
                        dst = qkT[:, ft, qc * QC:(qc + 1) * QC]
                    else:
                        dst = vT[:, ft - 6, qc * QC:(qc + 1) * QC]
                    nc.vector.tensor_copy(dst, ps[:])

        # ---- Phase C: attention per head ----
        with tc.tile_pool(name="vn", bufs=2) as vnp, \
             tc.tile_pool(name="esb", bufs=2) as esp, \
             tc.tile_pool(name="sums", bufs=2) as smp, \
             tc.tile_pool(name="rec", bufs=2) as rcp, \
             tc.tile_pool(name="ps_s", bufs=2, space="PSUM") as pss, \
             tc.tile_pool(name="ps_o", bufs=1, space="PSUM") as pso, \
             tc.tile_pool(name="ps_r", bufs=1, space="PSUM") as prc, \
             tc.tile_pool(name="ptr2", bufs=1, space="PSUM") as ptr2:
            for h in range(HL):
                base = (h % 2) * 64       # partition offset of this head
                oh = 64 - base            # start of the "ones" half
                ftq, ftk, ftv = h // 2, 3 + h // 2, h // 2
                vn = vnp.tile([128, KT, 128], f32, tag="vn")
                nc.vector.memset(vn[:, :, oh:oh + 64], 1.0)
                for kt in range(KT):
                    ps = ptr2.tile([128, 64], f32, tag="ptr2")
                    nc.tensor.transpose(
                        ps[:],
                        vT[base:base + 64, ftv, kt * 128:(kt + 1) * 128],
                        ident[base:base + 64, base:base + 64])
                    nc.vector.tensor_copy(vn[:, kt, base:base + 64], ps[:])
                for qc in range(NQC):
                    es = esp.tile([128, KT, QC], f32, tag="es")
                    es_flat = es[:].rearrange("p k q -> p (k q)")
                    for sg in range(NSW):
                        ps = pss.tile([128, SW], f32, tag="ps_s")
                        for j in range(KPG):
                            kt = sg * KPG + j
                            nc.tensor.matmul(
                                ps[:, j * QC:(j + 1) * QC],
                                qkT[base:base + 64, ftk,
                                    kt * 128:(kt + 1) * 128],
                                qkT[base:base + 64, ftq,
                                    qc * QC:(qc + 1) * QC],
                                start=True, stop=True)
                        nc.scalar.activation(
                            es_flat[:, sg * SW:(sg + 1) * SW], ps[:],
                            EXP, scale=SCALE)
                    po = pso.tile([128, QC], f32, tag="ps_o")
                    for kt in range(KT):
                        nc.tensor.matmul(
                            po[:], vn[:, kt, :], es[:, kt, :],
                            start=(kt == 0), stop=(kt == KT - 1))
                    sums = smp.tile([128, QC], f32, tag="sums")
                    rec = rcp.tile([128, QC], f32, tag="rec")
                    rec_ps = prc.tile([128, QC], f32, tag="ps_r")
                    nc.vector.tensor_copy(sums[oh:oh + 1, :], po[oh:oh + 1, :])
                    # replicate the sums row across 64 partitions: ones x sums
                    nc.tensor.matmul(
                        rec_ps[base:base + 64, :],
                        ones64[oh:oh + 1, :], sums[oh:oh + 1, :],
                        start=True, stop=True)
                    nc.vector.reciprocal(
                        rec[base:base + 64, :], rec_ps[base:base + 64, :])
                    nc.vector.tensor_mul(
                        oT[base:base + 64, ftq, qc * QC:(qc + 1) * QC],
                        po[base:base + 64, :], rec[base:base + 64, :])

        # ---- Phase D: output projection ----
        with tc.tile_pool(name="wp", bufs=1) as wpp, \
             tc.tile_pool(name="ysb", bufs=3) as ysp, \
             tc.tile_pool(name="ps_y", bufs=4, space="PSUM") as psy:
            wp_sb = wpp.tile([128, HL // 2, D], f32)
            nc.sync.dma_start(
                wp_sb[:], wp_d.rearrange("(ko ki) f -> ki ko f", ki=128))
            for tt in range(KT):
                ysb = ysp.tile([128, D], f32, tag="ysb")
                for nf, n0 in ((512, 0), (256, 512)):
                    ps = psy.tile([128, 512], f32, tag="ps_y")
                    for ks in range(HL // 2):
                        nc.tensor.matmul(
                            ps[:, :nf],
                            oT[:, ks, tt * 128:(tt + 1) * 128],
                            wp_sb[:, ks, n0:n0 + nf],
                            start=(ks == 0), stop=(ks == HL // 2 - 1))
                    nc.vector.tensor_copy(ysb[:, n0:n0 + nf], ps[:, :nf])
                nc.sync.dma_start(y_d[tt * 128:(tt + 1) * 128, :], ysb[:])

        if repeat > 1:
            rep_cm.__exit__(None, None, None)

    nc.compile()
    return nc


def _shard_inputs(x, w_qkv, w_proj):
    x = np.asarray(x, dtype=np.float32)
    w_qkv = np.asarray(w_qkv, dtype=np.float32)
    w_proj = np.asarray(w_proj, dtype=np.float32)
    in_maps = []
    for c in range(N_CORES):
        b, h0 = c // 2, (c % 2) * HL
        wq = np.concatenate(
            [w_qkv[:, t * D + h0 * HD: t * D + (h0 + HL) * HD]
             for t in range(3)], axis=1)
        wp = w_proj[h0 * HD:(h0 + HL) * HD, :]
        in_maps.append({
            "x": np.ascontiguousarray(x[b]),
            "w_qkv": np.ascontiguousarray(wq),
            "w_proj": np.ascontiguousarray(wp),
        })
    return in_maps


_NC_CACHE = {}


def kernel(x, w_qkv, w_proj, b_proj):
    from concourse.bass_utils import run_bass_kernel_spmd

    if "nc" not in _NC_CACHE:
        _NC_CACHE["nc"] = build_program()
    nc = _NC_CACHE["nc"]
    in_maps = _shard_inputs(x, w_qkv, w_proj)
    res = run_bass_kernel_spmd(nc, in_maps, core_ids=list(range(N_CORES)))
    b_proj = np.asarray(b_proj, dtype=np.float32)
    y = np.empty((B_FULL, N_FULL, D), np.float32)
    for b in range(B_FULL):
        y[b] = res.results[2 * b]["y"] + res.results[2 * b + 1]["y"] + b_proj
    return y


# revision 19
# speedup vs baseline: 1660.7928x; 1660.7928x over previous
"""Multi-head attention (B=4, N=2048, D=768, H=12) on 8 TRN2 NeuronCores.

Sharding: batch x head-group. Core c handles batch c//2, heads
[(c%2)*6, (c%2)*6+6). Each core computes qkv projection for its 6 heads
(column-sliced w_qkv), attention, and a partial output projection
(row-sliced w_proj). Host sums the two partial projections per batch and
adds the bias.

Per-core device dataflow (all fp32):
  A: x [NT,768] -> PE-transpose -> xT [128,6,NT] (dim-major)
  B: qkv^T = w_qkv_c^T-slices @ xT -> qkT [128,6,NT] (q:0-2,k:3-5), vT [128,3,NT]
  C: per head h (paired 2 heads per 128-partition tile, base=(h%2)*64):
       v_nat via PE-transpose -> vn [128,KT,128] (v cols at base-half, ones else)
       per q-chunk: S^T tiles = kT_h^T-slices @ qT_h chunk -> psum
                    exp(S*scale) -> es sbuf; O'^T = vn^T @ es (accum 16)
                    po rows base-half = O', other half = row-sums
                    normalize: copy 1 sums row -> sbuf, DMA-broadcast, recip, mul
  D: y = oT^T-slices @ w_proj_c -> psum -> sbuf -> DRAM y [NT,768]
"""

import numpy as np
from contextlib import ExitStack

D = 768
F = 1152          # 3 * 6 heads * 64 per core
HL = 6            # local heads per core
HD = 64
KO = D // 128     # 6 contraction slices for qkv
FT = F // 128     # 9 feature tiles
SCALE = HD ** -0.5
N_CORES = 8
B_FULL, N_FULL = 4, 2048

f32 = None  # set lazily (mybir import is heavy; keep module import cheap)


def build_program(NT=N_FULL, n_cores=N_CORES, repeat=1, use_f32r=True):
    import concourse.bacc as bacc
    import concourse.tile as tile
    import concourse.mybir as mybir
    from concourse.masks import make_identity

    f32 = mybir.dt.float32
    mdt = mybir.dt.float32r if use_f32r else mybir.dt.float32
    EXP = mybir.ActivationFunctionType.Exp

    KT = NT // 128            # token tiles
    QC = min(512, NT)         # q-chunk width
    NQC = NT // QC
    SW = min(1024, 2 * QC)    # S^T psum tile width (exp batch)
    KPG = SW // QC            # k-tiles per S psum group
    NSW = KT // KPG           # S psum groups per q-chunk

    nc = bacc.Bacc("TRN2", target_bir_lowering=False, debug=False,
                   enable_asserts=False, num_devices=n_cores)
    x_d = nc.dram_tensor("x", [NT, D], mdt, kind="ExternalInput").ap()
    wq_d = nc.dram_tensor("w_qkv", [D, F], mdt, kind="ExternalInput").ap()
    wp_d = nc.dram_tensor("w_proj", [HL * HD, D], mdt, kind="ExternalInput").ap()
    y_d = nc.dram_tensor("y", [NT, D], mdt, kind="ExternalOutput").ap()

    with tile.TileContext(nc) as tc, ExitStack() as ctx:
        constp = ctx.enter_context(tc.tile_pool(name="const", bufs=1))
        ident = constp.tile([128, 128], mdt)
        make_identity(nc, ident)
        ones64 = constp.tile([128, 64], mdt)
        nc.vector.memset(ones64[:], 1.0)

        actp = ctx.enter_context(tc.tile_pool(name="acts", bufs=1))
        qkT = actp.tile([128, 2 * (HL // 2), NT], mdt)   # q tiles 0-2, k tiles 3-5
        vT = actp.tile([128, HL // 2, NT], mdt)
        oT = actp.tile([128, HL // 2, NT], mdt)

        if repeat > 1:
            rep_cm = tc.For_i(0, repeat, 1)
            rep_cm.__enter__()

        # ---- Phase A: load + transpose x; Phase B: qkv^T ----
        with tc.tile_pool(name="xa", bufs=1) as xap, \
             tc.tile_pool(name="xload", bufs=3) as xlp, \
             tc.tile_pool(name="wq", bufs=1) as wqp, \
             tc.tile_pool(name="ptr", bufs=2, space="PSUM") as ptr, \
             tc.tile_pool(name="pqkv", bufs=4, space="PSUM") as pqkv:
            xT = xap.tile([128, KO, NT], mdt)
            wq_sb = wqp.tile([128, KO, F], mdt)
            nc.sync.dma_start(
                wq_sb[:], wq_d.rearrange("(ko ki) f -> ki ko f", ki=128))
            for tt in range(KT):
                xt = xlp.tile([128, D], mdt, tag="xload")
                nc.sync.dma_start(xt[:], x_d[tt * 128:(tt + 1) * 128, :])
                for ks in range(KO):
                    ps = ptr.tile([128, 128], mdt, tag="ptr")
                    nc.tensor.transpose(
                        ps[:], xt[:, ks * 128:(ks + 1) * 128], ident[:])
                    nc.vector.tensor_copy(
                        xT[:, ks, tt * 128:(tt + 1) * 128], ps[:])
            # order feature tiles so head-pair p's q (p), k (3+p) and v (6+p)
            # land before pair p+1's — lets attention start earlier
            ft_order = [p + 3 * g for p in range(3) for g in range(3)]
            for ft in ft_order:
                for qc in range(NQC):
                    ps = pqkv.tile([128, QC], f32, tag="pqkv")
                    for ks in range(KO):
                        nc.tensor.matmul(
                            ps[:],
                            wq_sb[:, ks, ft * 128:(ft + 1) * 128],
                            xT[:, ks, qc * QC:(qc + 1) * QC],
                            start=(ks == 0), stop=(ks == KO - 1))
                    if ft < 6:
                        dst = qkT[:, ft, qc * QC:(qc + 1) * QC]
                    else:
                        dst = vT[:, ft - 6, qc * QC:(qc + 1) * QC]
                    nc.vector.tensor_copy(dst, ps[:])

        # ---- Phase C: attention per head ----
        with tc.tile_pool(name="vn", bufs=2) as vnp, \
             tc.tile_pool(name="esb", bufs=2) as esp, \
             tc.tile_pool(name="sums", bufs=2) as smp, \
             tc.tile_pool(name="rec", bufs=2) as rcp, \
             tc.tile_pool(name="ps_s", bufs=3, space="PSUM") as pss, \
             tc.tile_pool(name="ps_o", bufs=2, space="PSUM") as pso:
            for h in range(HL):
                base = (h % 2) * 64       # partition offset of this head
                oh = 64 - base            # start of the "ones" half
                ftq, ftk, ftv = h // 2, 3 + h // 2, h // 2
                vn = vnp.tile([128, KT, 128], mdt, tag="vn")
                nc.vector.memset(vn[:, :, oh:oh + 64], 1.0)
                for kt in range(KT):
                    ps = pso.tile([128, QC], mdt, tag="ps_o")
                    nc.tensor.transpose(
                        ps[:, :64],
                        vT[base:base + 64, ftv, kt * 128:(kt + 1) * 128],
                        ident[base:base + 64, base:base + 64])
                    nc.vector.tensor_copy(vn[:, kt, base:base + 64], ps[:, :64])
                for qc in range(NQC):
                    es = esp.tile([128, KT, QC], mdt, tag="es")
                    es_flat = es[:].rearrange("p k q -> p (k q)")
                    for sg in range(NSW):
                        ps = pss.tile([128, SW], f32, tag="ps_s")
                        for j in range(KPG):
                            kt = sg * KPG + j
                            nc.tensor.matmul(
                                ps[:, j * QC:(j + 1) * QC],
                                qkT[base:base + 64, ftk,
                                    kt * 128:(kt + 1) * 128],
                                qkT[base:base + 64, ftq,
                                    qc * QC:(qc + 1) * QC],
                                start=True, stop=True)
                        nc.scalar.activation(
                            es_flat[:, sg * SW:(sg + 1) * SW], ps[:],
                            EXP, scale=SCALE)
                    po = pso.tile([128, QC], f32, tag="ps_o")
                    for kt in range(KT):
                        nc.tensor.matmul(
                            po[:], vn[:, kt, :], es[:, kt, :],
                            start=(kt == 0), stop=(kt == KT - 1))
                    sums = smp.tile([128, QC], mdt, tag="sums")
                    rec = rcp.tile([128, QC], f32, tag="rec")
                    rec_ps = pss.tile([128, SW], f32, tag="ps_s")
                    nc.vector.tensor_copy(sums[oh:oh + 1, :], po[oh:oh + 1, :])
                    # replicate the sums row across 64 partitions: ones x sums
                    nc.tensor.matmul(
                        rec_ps[base:base + 64, :QC],
                        ones64[oh:oh + 1, :], sums[oh:oh + 1, :],
                        start=True, stop=True)
                    nc.vector.reciprocal(
                        rec[base:base + 64, :], rec_ps[base:base + 64, :QC])
                    nc.vector.tensor_mul(
                        oT[base:base + 64, ftq, qc * QC:(qc + 1) * QC],
                        po[base:base + 64, :], rec[base:base + 64, :])

        # ---- Phase D: output projection ----
        with tc.tile_pool(name="wp", bufs=1) as wpp, \
             tc.tile_pool(name="ysb", bufs=3) as ysp, \
             tc.tile_pool(name="ps_y", bufs=4, space="PSUM") as psy:
            wp_sb = wpp.tile([128, HL // 2, D], mdt)
            nc.sync.dma_start(
                wp_sb[:], wp_d.rearrange("(ko ki) f -> ki ko f", ki=128))
            for tt in range(KT):
                ysb = ysp.tile([128, D], mdt, tag="ysb")
                for nf, n0 in ((512, 0), (256, 512)):
                    ps = psy.tile([128, 512], f32, tag="ps_y")
                    for ks in range(HL // 2):
                        nc.tensor.matmul(
                            ps[:, :nf],
                            oT[:, ks, tt * 128:(tt + 1) * 128],
                            wp_sb[:, ks, n0:n0 + nf],
                            start=(ks == 0), stop=(ks == HL // 2 - 1))
                    nc.vector.tensor_copy(ysb[:, n0:n0 + nf], ps[:, :nf])
                nc.sync.dma_start(y_d[tt * 128:(tt + 1) * 128, :], ysb[:])

        if repeat > 1:
            rep_cm.__exit__(None, None, None)

    nc.compile()
    return nc


def _shard_inputs(x, w_qkv, w_proj):
    x = np.asarray(x, dtype=np.float32)
    w_qkv = np.asarray(w_qkv, dtype=np.float32)
    w_proj = np.asarray(w_proj, dtype=np.float32)
    in_maps = []
    for c in range(N_CORES):
        b, h0 = c // 2, (c % 2) * HL
        wq = np.concatenate(
            [w_qkv[:, t * D + h0 * HD: t * D + (h0 + HL) * HD]
             for t in range(3)], axis=1)
        wp = w_proj[h0 * HD:(h0 + HL) * HD, :]
        in_maps.append({
            "x": np.ascontiguousarray(x[b]),
            "w_qkv": np.ascontiguousarray(wq),
            "w_proj": np.ascontiguousarray(wp),
        })
    return in_maps


_NC_CACHE = {}


def kernel(x, w_qkv, w_proj, b_proj):
    from concourse.bass_utils import run_bass_kernel_spmd

    if "nc" not in _NC_CACHE:
        _NC_CACHE["nc"] = build_program()
    nc = _NC_CACHE["nc"]
    in_maps = _shard_inputs(x, w_qkv, w_proj)
    res = run_bass_kernel_spmd(nc, in_maps, core_ids=list(range(N_CORES)))
    b_proj = np.asarray(b_proj, dtype=np.float32)
    y = np.empty((B_FULL, N_FULL, D), np.float32)
    for b in range(B_FULL):
        y[b] = res.results[2 * b]["y"] + res.results[2 * b + 1]["y"] + b_proj
    return y


# revision 22
# speedup vs baseline: 5162.1415x; 3.1082x over previous
"""Multi-head attention (B=4, N=2048, D=768, H=12) on 8 TRN2 NeuronCores.

Sharding: batch x head-group. Core c handles batch c//2, heads
[(c%2)*6, (c%2)*6+6). Each core computes qkv projection for its 6 heads
(column-sliced w_qkv), attention, and a partial output projection
(row-sliced w_proj). Host sums the two partial projections per batch and
adds the bias.

Per-core device dataflow (all fp32):
  A: x [NT,768] -> PE-transpose -> xT [128,6,NT] (dim-major)
  B: qkv^T = w_qkv_c^T-slices @ xT -> qkT [128,6,NT] (q:0-2,k:3-5), vT [128,3,NT]
  C: per head h (paired 2 heads per 128-partition tile, base=(h%2)*64):
       v_nat via PE-transpose -> vn [128,KT,128] (v cols at base-half, ones else)
       per q-chunk: S^T tiles = kT_h^T-slices @ qT_h chunk -> psum
                    exp(S*scale) -> es sbuf; O'^T = vn^T @ es (accum 16)
                    po rows base-half = O', other half = row-sums
                    normalize: copy 1 sums row -> sbuf, DMA-broadcast, recip, mul
  D: y = oT^T-slices @ w_proj_c -> psum -> sbuf -> DRAM y [NT,768]
"""

import numpy as np
from contextlib import ExitStack

D = 768
F = 1152          # 3 * 6 heads * 64 per core
HL = 6            # local heads per core
HD = 64
KO = D // 128     # 6 contraction slices for qkv
FT = F // 128     # 9 feature tiles
SCALE = HD ** -0.5
N_CORES = 8
B_FULL, N_FULL = 4, 2048

f32 = None  # set lazily (mybir import is heavy; keep module import cheap)


def build_program(NT=N_FULL, n_cores=N_CORES, repeat=1, use_f32r=True):
    import concourse.bacc as bacc
    import concourse.tile as tile
    import concourse.mybir as mybir
    from concourse.masks import make_identity

    f32 = mybir.dt.float32
    mdt = mybir.dt.float32r if use_f32r else mybir.dt.float32
    EXP = mybir.ActivationFunctionType.Exp

    KT = NT // 128            # token tiles
    QC = min(512, NT)         # q-chunk width
    NQC = NT // QC
    SW = min(1024, 2 * QC)    # S^T psum tile width (exp batch)
    KPG = SW // QC            # k-tiles per S psum group
    NSW = KT // KPG           # S psum groups per q-chunk

    nc = bacc.Bacc("TRN2", target_bir_lowering=False, debug=False,
                   enable_asserts=False, num_devices=n_cores)
    x_d = nc.dram_tensor("x", [NT, D], mdt, kind="ExternalInput").ap()
    wq_d = nc.dram_tensor("w_qkv", [D, F], mdt, kind="ExternalInput").ap()
    wp_d = nc.dram_tensor("w_proj", [HL * HD, D], mdt, kind="ExternalInput").ap()
    y_d = nc.dram_tensor("y", [NT, D], f32, kind="ExternalOutput").ap()

    with tile.TileContext(nc) as tc, ExitStack() as ctx:
        constp = ctx.enter_context(tc.tile_pool(name="const", bufs=1))
        ident_f = constp.tile([128, 128], f32)
        make_identity(nc, ident_f)
        ident = constp.tile([128, 128], mdt)
        nc.vector.tensor_copy(ident[:], ident_f[:])
        ones_f = constp.tile([128, 64], f32)
        nc.vector.memset(ones_f[:], 1.0)
        ones64 = constp.tile([128, 64], mdt)
        nc.vector.tensor_copy(ones64[:], ones_f[:])

        actp = ctx.enter_context(tc.tile_pool(name="acts", bufs=1))
        qkT = actp.tile([128, 2 * (HL // 2), NT], mdt)   # q tiles 0-2, k tiles 3-5
        vT = actp.tile([128, HL // 2, NT], mdt)
        oT = actp.tile([128, HL // 2, NT], mdt)

        if repeat > 1:
            rep_cm = tc.For_i(0, repeat, 1)
            rep_cm.__enter__()

        # ---- Phase A: load + transpose x; Phase B: qkv^T ----
        with tc.tile_pool(name="xa", bufs=1) as xap, \
             tc.tile_pool(name="xload", bufs=3) as xlp, \
             tc.tile_pool(name="wq", bufs=1) as wqp, \
             tc.tile_pool(name="ptr", bufs=2, space="PSUM") as ptr, \
             tc.tile_pool(name="pqkv", bufs=4, space="PSUM") as pqkv:
            xT = xap.tile([128, KO, NT], mdt)
            wq_sb = wqp.tile([128, KO, F], mdt)
            nc.sync.dma_start(
                wq_sb[:], wq_d.rearrange("(ko ki) f -> ki ko f", ki=128))
            for tt in range(KT):
                xt = xlp.tile([128, D], mdt, tag="xload")
                nc.sync.dma_start(xt[:], x_d[tt * 128:(tt + 1) * 128, :])
                for ks in range(KO):
                    ps = ptr.tile([128, 128], mdt, tag="ptr")
                    nc.tensor.transpose(
                        ps[:], xt[:, ks * 128:(ks + 1) * 128], ident[:])
                    nc.vector.tensor_copy(
                        xT[:, ks, tt * 128:(tt + 1) * 128], ps[:])
            # order feature tiles so head-pair p's q (p), k (3+p) and v (6+p)
            # land before pair p+1's — lets attention start earlier
            ft_order = [p + 3 * g for p in range(3) for g in range(3)]
            for ft in ft_order:
                for qc in range(NQC):
                    ps = pqkv.tile([128, QC], f32, tag="pqkv")
                    for ks in range(KO):
                        nc.tensor.matmul(
                            ps[:],
                            wq_sb[:, ks, ft * 128:(ft + 1) * 128],
                            xT[:, ks, qc * QC:(qc + 1) * QC],
                            start=(ks == 0), stop=(ks == KO - 1))
                    if ft < 6:
                        dst = qkT[:, ft, qc * QC:(qc + 1) * QC]
                    else:
                        dst = vT[:, ft - 6, qc * QC:(qc + 1) * QC]
                    nc.vector.tensor_copy(dst, ps[:])

        # ---- Phase C: attention per head ----
        with tc.tile_pool(name="vn", bufs=2) as vnp, \
             tc.tile_pool(name="esb", bufs=2) as esp, \
             tc.tile_pool(name="sums", bufs=2) as smp, \
             tc.tile_pool(name="rec", bufs=2) as rcp, \
             tc.tile_pool(name="recb", bufs=2) as rcbp, \
             tc.tile_pool(name="drs", bufs=2, space="DRAM") as drp, \
             tc.tile_pool(name="ps_s", bufs=3, space="PSUM") as pss, \
             tc.tile_pool(name="ps_o", bufs=2, space="PSUM") as pso:
            for h in range(HL):
                base = (h % 2) * 64       # partition offset of this head
                oh = 64 - base            # start of the "ones" half
                ftq, ftk, ftv = h // 2, 3 + h // 2, h // 2
                vn = vnp.tile([128, KT, 128], mdt, tag="vn")
                nc.vector.tensor_copy(
                    vn[:, :, oh:oh + 64],
                    ones64[:, None, :].to_broadcast((128, KT, 64)))
                for kt in range(KT):
                    ps = pso.tile([128, QC], mdt, tag="ps_o")
                    nc.tensor.transpose(
                        ps[:, :64],
                        vT[base:base + 64, ftv, kt * 128:(kt + 1) * 128],
                        ident[base:base + 64, base:base + 64])
                    nc.vector.tensor_copy(vn[:, kt, base:base + 64], ps[:, :64])
                for qc in range(NQC):
                    es = esp.tile([128, KT, QC], mdt, tag="es")
                    es_flat = es[:].rearrange("p k q -> p (k q)")
                    for sg in range(NSW):
                        ps = pss.tile([128, SW], f32, tag="ps_s")
                        for j in range(KPG):
                            kt = sg * KPG + j
                            nc.tensor.matmul(
                                ps[:, j * QC:(j + 1) * QC],
                                qkT[base:base + 64, ftk,
                                      kt * 128:(kt + 1) * 128],
                                qkT[base:base + 64, ftq,
                                      qc * QC:(qc + 1) * QC],
                                start=True, stop=True)
                        nc.scalar.activation(
                            es_flat[:, sg * SW:(sg + 1) * SW], ps[:],
                            EXP, scale=SCALE)
                    po = pso.tile([128, QC], f32, tag="ps_o")
                    for kt in range(KT):
                        nc.tensor.matmul(
                            po[:], vn[:, kt, :], es[:, kt, :],
                            start=(kt == 0), stop=(kt == KT - 1))
                    sums = smp.tile([128, QC], f32, tag="sums")
                    rec = rcp.tile([128, QC], f32, tag="rec")
                    recb = rcbp.tile([128, QC], f32, tag="recb")
                    srow = drp.tile([1, QC], f32, tag="srow")
                    nc.vector.tensor_copy(sums[oh:oh + 1, :], po[oh:oh + 1, :])
                    # replicate the sums row across 64 partitions via DRAM
                    nc.sync.dma_start(srow[:], sums[oh:oh + 1, :])
                    nc.sync.dma_start(
                        rec[base:base + 64, :],
                        srow[:].to_broadcast((64, QC)))
                    nc.vector.reciprocal(
                        recb[base:base + 64, :], rec[base:base + 64, :])
                    nc.vector.tensor_mul(
                        oT[base:base + 64, ftq, qc * QC:(qc + 1) * QC],
                        po[base:base + 64, :], recb[base:base + 64, :])

        # ---- Phase D: output projection ----
        with tc.tile_pool(name="wp", bufs=1) as wpp, \
             tc.tile_pool(name="ysb", bufs=3) as ysp, \
             tc.tile_pool(name="ps_y", bufs=4, space="PSUM") as psy:
            wp_sb = wpp.tile([128, HL // 2, D], mdt)
            nc.sync.dma_start(
                wp_sb[:], wp_d.rearrange("(ko ki) f -> ki ko f", ki=128))
            for tt in range(KT):
                ysb = ysp.tile([128, D], f32, tag="ysb")
                for nf, n0 in ((512, 0), (256, 512)):
                    ps = psy.tile([128, 512], f32, tag="ps_y")
                    for ks in range(HL // 2):
                        nc.tensor.matmul(
                            ps[:, :nf],
                            oT[:, ks, tt * 128:(tt + 1) * 128],
                            wp_sb[:, ks, n0:n0 + nf],
                            start=(ks == 0), stop=(ks == HL // 2 - 1))
                    nc.vector.tensor_copy(ysb[:, n0:n0 + nf], ps[:, :nf])
                nc.sync.dma_start(y_d[tt * 128:(tt + 1) * 128, :], ysb[:])

        if repeat > 1:
            rep_cm.__exit__(None, None, None)

    nc.compile()
    return nc


def _shard_inputs(x, w_qkv, w_proj):
    x = np.asarray(x, dtype=np.float32)
    w_qkv = np.asarray(w_qkv, dtype=np.float32)
    w_proj = np.asarray(w_proj, dtype=np.float32)
    in_maps = []
    for c in range(N_CORES):
        b, h0 = c // 2, (c % 2) * HL
        wq = np.concatenate(
            [w_qkv[:, t * D + h0 * HD: t * D + (h0 + HL) * HD]
             for t in range(3)], axis=1)
        wp = w_proj[h0 * HD:(h0 + HL) * HD, :]
        in_maps.append({
            "x": np.ascontiguousarray(x[b]),
            "w_qkv": np.ascontiguousarray(wq),
            "w_proj": np.ascontiguousarray(wp),
        })
    return in_maps


_NC_CACHE = {}


def kernel(x, w_qkv, w_proj, b_proj):
    from concourse.bass_utils import run_bass_kernel_spmd

    if "nc" not in _NC_CACHE:
        _NC_CACHE["nc"] = build_program()
    nc = _NC_CACHE["nc"]
    in_maps = _shard_inputs(x, w_qkv, w_proj)
    res = run_bass_kernel_spmd(nc, in_maps, core_ids=list(range(N_CORES)))
    b_proj = np.asarray(b_proj, dtype=np.float32)
    y = np.empty((B_FULL, N_FULL, D), np.float32)
    for b in range(B_FULL):
        y[b] = res.results[2 * b]["y"] + res.results[2 * b + 1]["y"] + b_proj
    return y
